# revision 1
# baseline (speedup 1.0000x reference)
"""FAVOR+ causal linear attention (relu feature map) on 8 Trainium2 NeuronCores.

Data-parallel over batch: B=8 batch elements -> one per core. Per core, a
sequence-chunked scan (16 chunks of 128 positions) with an (M x V+1) running
state (SBUF, DVE add-chain) implements the causal prefix-sum attention:

  phi = relu(x @ W) + eps
  out[l] = phi_q[l] @ (sum_{l'<=l} phi_k[l'] (x) v[l']) / (phi_q[l] . sum phi_k)

Chunk recurrence (C=128):  A^T = phi_kT^T phi_qT (masked upper-tri);
  out_chunk(C,V+1) = phi_qT^T @ S_aug + A_masked^T^T @ V_aug ; S_aug += phi_k^T V_aug
where the +1 column carries the normalizer (ones-augmented values / z-state).
All matmuls fp32 (exact-fp32 two-pass PE mode).

Host-side layout tricks: values are fed pre-scrambled into the device SBUF
layout (2, 128, 8*(V+1)) with the ones-column baked in, and the output is
written in device layout (2, 128, 8*V) and unscrambled on the host — every
DMA is fully contiguous and all on-chip V/output transposes disappear.
SBUF partition half h = i//8 holds sequence half h; phi is produced in
pipelined (64,512) pieces feeding the per-chunk scan.

Quirks worked around (this walrus/axon container): one sync-wait per
instruction (waits split onto NoOps post-lowering); PSUM banks must not mix
concurrent PE writes + engine reads on disjoint regions of one bank (HW
crash); tile_position row-tiling with fp32 matmuls is fatal on HW.
"""

import numpy as np

import concourse.bass as bass
import concourse.mybir as mybir
from concourse.tile import TileContext
from concourse.bass_utils import run_bass_kernel_spmd
from bass_rust import ScopedClock, VectorClock

f32 = mybir.dt.float32
f32r = mybir.dt.float32r

# When True, the phi projection matmuls run in float32r (TF32-like, 4x faster
# on PE at N>=512). Adds ~1e-4 scale-relative error to phi; everything
# downstream stays exact fp32.
PHI_F32R = False

B, D, L, M, V = 8, 64, 2048, 64, 64
KERNEL_EPS = 0.001
C = 128          # chunk length
NCH = L // C     # 16 chunks
NCORES = 8

LABELS = {}      # instruction name -> semantic label (for sim profiling)


def _lab(label, bi):
    LABELS[bi.ins.name] = label
    return bi


class _TileContextSplitDrain(TileContext):
    """This walrus build allows only ONE sync-wait command per instruction.
    Split the exit drain's waits into single-wait nops."""

    def _drain_and_barrier(self, tick_clock, wait_clock):
        from concourse.tile_scheduler import PROC_NAME_TO_IDX

        gc = tick_clock.global_clock
        ticks = list(gc)
        n = len(ticks)
        keep = set()
        for name, idx in PROC_NAME_TO_IDX.items():
            if name in ("PE", "DVE", "Activation", "SP", "Pool"):
                keep.add(idx)
        for inst in getattr(self.nc, "_tail_insts", []):
            p = inst.bass_scheduled_proc
            if p is not None:
                keep.add(p)
        for j in range(n):
            if ticks[j] <= 0 or j not in keep:
                continue
            vec = [0] * n
            vec[j] = ticks[j]
            nop = self.nc.sync.nop(nofuse=True, hint="split_drain_wait")
            wait_clock.add_sem_waits(nop.ins, ScopedClock({None: VectorClock(vec)}))
        self.nc.sync.drain()
        self.nc.all_engine_barrier()
        assert self.sems is not None
        popped = self.nc._tile_sem_poison_stack.pop()
        assert popped is self._sem_poison
        self.nc.clear_and_free_semaphores(list(self.sems.allocated().values()))
        self.nc.all_engine_barrier()


def _split_instruction_waits(nc):
    """Move excess sem waits (>1) onto same-engine NoOps inserted just before
    the instruction; the sequencer executes them in order, so semantics are
    unchanged."""
    counter = 0
    for f in nc.m.functions:
        for bb in f.blocks:
            il = list(bb.instructions)
            out = []
            changed = False
            for inst in il:
                si = inst.sync_info
                if si is not None and si.on_wait and len(si.on_wait) > 1:
                    waits = list(si.on_wait)
                    extra, keep = waits[:-1], waits[-1:]
                    for w in extra:
                        nop = mybir.InstNoOp(
                            name=f"waitsplit-{counter}", engine=inst.engine,
                            ins=[], outs=[],
                            sync_info=mybir.SyncInfo(on_wait=[w], on_update=[]))
                        counter += 1
                        out.append(nop)
                    si.on_wait = keep
                    inst.sync_info = si
                    changed = True
                out.append(inst)
            if changed:
                bb.instructions = out
    return counter


def build(repeats: int = 1, split_waits: bool = True) -> bass.Bass:
    LABELS.clear()
    fphi = f32r if PHI_F32R else f32
    nc = bass.Bass()
    keys_d = nc.dram_tensor("keys", [D, L], fphi, kind="ExternalInput")
    valt_d = nc.dram_tensor("valt", [2, 128, 8 * (V + 1)], f32, kind="ExternalInput")
    queries_d = nc.dram_tensor("queries", [D, L], fphi, kind="ExternalInput")
    proj_d = nc.dram_tensor("proj", [D, M], fphi, kind="ExternalInput")
    ident_d = nc.dram_tensor("ident", [64, 64], f32, kind="ExternalInput")
    mask_d = nc.dram_tensor("masku", [C, C], f32, kind="ExternalInput")
    outt_d = nc.dram_tensor("outt", [2, 128, 8 * V], f32, kind="ExternalOutput")

    mx = mybir.AluOpType.max
    ad = mybir.AluOpType.add
    ml = mybir.AluOpType.mult
    actCopy = mybir.ActivationFunctionType.Copy

    nc._tail_insts = []

    with _TileContextSplitDrain(nc) as tc:
        with (
            tc.tile_pool(name="const", bufs=1) as const,
            tc.tile_pool(name="io", bufs=2) as io,
            tc.tile_pool(name="psPhi", bufs=1, space="PSUM") as psPhi,
            tc.tile_pool(name="psA", bufs=2, space="PSUM") as psA,
            tc.tile_pool(name="psK", bufs=2, space="PSUM") as psK,
            tc.tile_pool(name="psS", bufs=1, space="PSUM") as psS,
            tc.tile_pool(name="psO", bufs=2, space="PSUM") as psO,
            tc.tile_pool(name="sb3", bufs=3) as sb3,
            tc.tile_pool(name="sb2", bufs=2) as sb2,
        ):
            w_s = const.tile([128, M], fphi, tag="w")

            for _ in range(repeats):
                # ---- inputs
                ins = {}
                for nm in ("k", "q"):
                    t = io.tile([128, 1024], fphi, tag=f"in_{nm}")
                    ins[nm] = t
                # V^T halves, 65-strided chunk blocks (col 64 = ones)
                vts = {}
                for h in range(2):
                    t = io.tile([128, 8 * (V + 1)], f32, tag=f"in_v{h}")
                    vts[h] = t
                dmap = {"k": keys_d, "q": queries_d}
                _lab("dma_k0a", nc.sync.dma_start(
                    ins["k"][0:64, 0:512], keys_d[:, 0:512]))
                _lab("dma_w0", nc.gpsimd.dma_start(w_s[0:64, :], proj_d[:]))
                _lab("dma_q0a", nc.sync.dma_start(
                    ins["q"][0:64, 0:512], queries_d[:, 0:512]))
                for nm in ("k", "q"):
                    _lab(f"dma_{nm}0b", nc.sync.dma_start(
                        ins[nm][0:64, 512:1024], dmap[nm][:, 512:1024]))
                id_s = const.tile([64, 64], f32, tag="id")
                _lab("dma_id", nc.gpsimd.dma_start(id_s[:], ident_d[:]))
                mk_s = const.tile([C, C], f32, tag="mk")
                _lab("dma_mk", nc.gpsimd.dma_start(mk_s[:], mask_d[:]))
                for h in range(2):
                    _lab(f"dma_v{h}", nc.sync.dma_start(vts[h][:], valt_d[h]))
                _lab("dma_w1", nc.gpsimd.dma_start(w_s[64:128, :], proj_d[:]))
                for nm in ("k", "q"):
                    _lab(f"dma_{nm}1", nc.sync.dma_start(
                        ins[nm][64:128, :], dmap[nm][:, 1024:2048]))

                # ---- output staging: (128, 8*64) per half, (L, V) layout
                o_half = {}
                for h in range(2):
                    oh = io.tile([128, 8 * V], f32, tag=f"out{h}")
                    o_half[h] = oh

                phiqk = {}   # h -> sbuf tile (64, 2048): [q 1024 | k 1024]
                
                S_cur = None
                for i in range(NCH):
                    h, a = i // 8, i % 8
                    rows = slice(64 * h, 64 * h + 64)
                    wh = w_s[rows, :]

                    # ---- half-batched phi, pipelined (64,512) pieces, k first
                    if h not in phiqk:
                        qk = sb2.tile([M, 2048], f32, tag="phiqk")
                        for u in range(2):
                            for g, nm in ((1, "k"), (0, "q")):
                                pphi = psPhi.tile([M, 512], f32, tag="phi")
                                _lab(f"mm_phi_{nm}{h}{u}", nc.tensor.matmul(
                                    pphi[:], lhsT=wh,
                                    rhs=ins[nm][rows, 512 * u:512 * u + 512],
                                    start=True, stop=True))
                                _lab(f"relu_{nm}{h}{u}", nc.vector.tensor_scalar(
                                    qk[:, 1024 * g + 512 * u:1024 * g + 512 * u + 512],
                                    pphi[:], 0.0, KERNEL_EPS, op0=mx, op1=ad))
                        phiqk[h] = qk
                    qk = phiqk[h]
                    phi_qT = qk[:, 128 * a:128 * a + 128]
                    phi_kT = qk[:, 1024 + 128 * a:1024 + 128 * a + 128]
                    Vt = vts[h][:, (V + 1) * a:(V + 1) * a + V + 1]

                    # ---- A^T (C, C) masked upper-tri (incl diag)
                    pA = psA.tile([C, C], f32, tag="A")
                    _lab(f"mm_A{i}", nc.tensor.matmul(
                        pA[:], lhsT=phi_kT, rhs=phi_qT, start=True, stop=True))
                    At = sb3.tile([C, C], f32, tag="At")
                    _lab(f"mask{i}", nc.vector.tensor_tensor(
                        At[:], pA[:], mk_s[:], op=ml))

                    # ---- phi_k (C, M) via PE transpose of phi_kT
                    pK = psK.tile([C, M], f32, tag="K")
                    _lab(f"trK{i}", nc.tensor.transpose(pK[:], phi_kT, id_s[:]))
                    phi_k = sb3.tile([C, M], f32, tag="phikc")
                    _lab(f"cpK{i}", nc.scalar.copy(phi_k[:], pK[:]))

                    # ---- state delta (PSUM) and SBUF state chain (DVE add)
                    dS = psS.tile([M, V + 1], f32, tag="dS")
                    _lab(f"mm_dS{i}", nc.tensor.matmul(
                        dS[:], lhsT=phi_k[:], rhs=Vt, start=True, stop=True))
                    S_next = sb3.tile([M, V + 1], f32, tag="S")
                    if i == 0:
                        _lab(f"Scp{i}", nc.vector.tensor_copy(S_next[:], dS[:]))
                    else:
                        _lab(f"Sadd{i}", nc.vector.tensor_tensor(
                            S_next[:], S_cur[:], dS[:], op=ad))

                    # ---- out chunk (C, V+1) = inter + intra
                    pO = psO.tile([C, V + 1], f32, tag="O")
                    if i > 0:
                        _lab(f"mm_inter{i}", nc.tensor.matmul(
                            pO[:], lhsT=phi_qT, rhs=S_cur[:],
                            start=True, stop=False))
                    _lab(f"mm_intra{i}", nc.tensor.matmul(
                        pO[:], lhsT=At[:], rhs=Vt,
                        start=(i == 0), stop=True))
                    S_cur = S_next

                    # ---- divide by normalizer column straight into staging
                    rec = sb3.tile([C, 1], f32, tag="rec")
                    _lab(f"recip{i}", nc.vector.reciprocal(rec[:], pO[:, V:V + 1]))
                    _lab(f"div{i}", nc.scalar.activation(
                        o_half[h][:, V * a:V * a + V], pO[:, 0:V], actCopy,
                        scale=rec[:, 0:1]))

                    # ---- flush output half (3 pieces; small final piece)
                    if a in (3, 5, 7):
                        lo = 0 if a == 3 else (a - 1) * V
                        hi = (a + 1) * V
                        eng = nc.sync if a == 7 else nc.gpsimd
                        di = _lab(f"dma_out{h}{a}", eng.dma_start(
                            outt_d[h, :, lo:hi], o_half[h][:, lo:hi]))
                        nc._tail_insts.append(di.ins)

    if split_waits:
        _split_instruction_waits(nc)
    return nc


_CONSTS = None


def _consts():
    global _CONSTS
    if _CONSTS is None:
        ident = np.eye(64, dtype=np.float32)
        masku = np.triu(np.ones((C, C), dtype=np.float32))
        _CONSTS = (ident, masku)
    return _CONSTS


def kernel(keys, values, queries, proj_matrix):
    keys = np.ascontiguousarray(keys, dtype=np.float32)
    queries = np.ascontiguousarray(queries, dtype=np.float32)
    proj_matrix = np.ascontiguousarray(proj_matrix, dtype=np.float32)
    vT = np.asarray(values, dtype=np.float32).transpose(0, 2, 1)  # (B, L, V)
    vT = vT.reshape(B, 2, 8, 128, V).transpose(0, 1, 3, 2, 4)  # (B,2,128,8,V)
    valt = np.ones((B, 2, 128, 8, V + 1), dtype=np.float32)
    valt[..., 0:V] = vT
    valt = np.ascontiguousarray(valt.reshape(B, 2, 128, 8 * (V + 1)))
    ident, masku = _consts()

    nc = build()
    in_maps = [
        {
            "keys": keys[b], "valt": valt[b], "queries": queries[b],
            "proj": proj_matrix, "ident": ident, "masku": masku,
        }
        for b in range(B)
    ]
    res = run_bass_kernel_spmd(nc, in_maps, list(range(NCORES)))
    # outt: (2, 128, 8*V) device layout -> out (V, L): out[v, 1024h+128a+p]
    outs = []
    for b in range(B):
        ot = res.results[b]["outt"].reshape(2, 128, 8, V)
        outs.append(ot.transpose(3, 0, 2, 1).reshape(V, L))
    return np.ascontiguousarray(np.stack(outs, axis=0), dtype=np.float32)


if __name__ == "__main__":
    rng = np.random.default_rng(0)
    ks = rng.standard_normal((B, D, L), dtype=np.float32)
    vs = rng.standard_normal((B, V, L), dtype=np.float32)
    qs = rng.standard_normal((B, D, L), dtype=np.float32)
    pm = np.linalg.qr(rng.standard_normal((D, M)))[0].astype(np.float32)
    o = kernel(ks, vs, qs, pm)
    print("kernel output", o.shape, o.dtype)



# revision 11
# speedup vs baseline: 1.3917x; 1.3917x over previous
"""FAVOR+ causal linear attention (relu feature map) on 8 Trainium2 NeuronCores.

Data-parallel over batch: B=8 -> one batch element per core. Per core, a
sequence-chunked scan (16 chunks of 128) with an (M x V+1) running state
implements the causal prefix-sum attention:

  phi = relu(x @ W) + eps
  out[l] = phi_q[l] @ (sum_{l'<=l} phi_k[l'] (x) v[l']) / (phi_q[l] . sum phi_k)

v2 design (vs the fp32 v1): everything on the PE runs in bf16 (1 cycle/row
instead of fp32's 4). Host packs W into the keys DMA ([W | k-half] rows),
values arrive pre-scrambled with a ones-column ((2,128,8*65) bf16), and the
normalizer DIVISION happens on the host: the kernel ships out (num | norm)
quad blocks, so on-chip there is no reciprocal / scale step at all.

Per quad of 4 chunks: 4 Kc matmuls (k_chunk^T @ W -> phi_k in (C,M) layout,
replacing v1's PE-transpose+copy), one Act relu over the quad; 4 A^T matmuls
into one PSUM bank, ONE DVE mask-multiply over (128,512); per chunk: dS
matmul, state add (Pool), inter+intra matmuls into a quad pO bank; one Act
copy (128,260) -> bf16 staging per quad, flushed by HWDGE DMA. PSUM banks:
shared {phi-pieces, A-quads} pool 4 + Kc 1 + dS 2 + pO 1 = 8.

relu placement: exact (max,add) tensor_scalar on Pool/DVE for h0 pieces;
Act relu(x+eps) approximation (abs err <= eps) for h1 pieces + phi_k quads.

Quirks worked around (this walrus/axon container): one sync-wait per
instruction (waits split onto NoOps post-lowering); PSUM banks must not mix
concurrent PE writes + engine reads on disjoint regions of one bank (HW
crash); tile_position row-tiling with fp32 matmuls is fatal on HW.
"""

import numpy as np

import concourse.bass as bass
import concourse.mybir as mybir
from concourse.tile import TileContext
from concourse.bass_utils import run_bass_kernel_spmd
from bass_rust import ScopedClock, VectorClock

f32 = mybir.dt.float32
bf16 = mybir.dt.bfloat16

B, D, L, M, V = 8, 64, 2048, 64, 64
KERNEL_EPS = 0.001
C = 128          # chunk length
NCH = L // C     # 16 chunks
NQ = NCH // 4    # 4 quads
NCORES = 8

LABELS = {}      # instruction name -> semantic label (for sim profiling)

# bisect flags
ACT_RELU = True      # Act relu(x+eps) approx for h1/phi_k (else DVE exact)
HOST_DIV = True      # ship norm column, divide on host


def _lab(label, bi):
    LABELS[bi.ins.name] = label
    return bi


class _TileContextSplitDrain(TileContext):
    """This walrus build allows only ONE sync-wait command per instruction.
    Split the exit drain's waits into single-wait nops."""

    def _drain_and_barrier(self, tick_clock, wait_clock):
        from concourse.tile_scheduler import PROC_NAME_TO_IDX

        gc = tick_clock.global_clock
        ticks = list(gc)
        n = len(ticks)
        keep = set()
        for name, idx in PROC_NAME_TO_IDX.items():
            if name in ("PE", "DVE", "Activation", "SP", "Pool"):
                keep.add(idx)
        for inst in getattr(self.nc, "_tail_insts", []):
            p = inst.bass_scheduled_proc
            if p is not None:
                keep.add(p)
        for j in range(n):
            if ticks[j] <= 0 or j not in keep:
                continue
            vec = [0] * n
            vec[j] = ticks[j]
            nop = self.nc.sync.nop(nofuse=True, hint="split_drain_wait")
            wait_clock.add_sem_waits(nop.ins, ScopedClock({None: VectorClock(vec)}))
        self.nc.sync.drain()
        self.nc.all_engine_barrier()
        assert self.sems is not None
        popped = self.nc._tile_sem_poison_stack.pop()
        assert popped is self._sem_poison
        self.nc.clear_and_free_semaphores(list(self.sems.allocated().values()))
        self.nc.all_engine_barrier()


def _split_instruction_waits(nc):
    """Move excess sem waits (>1) onto same-engine NoOps inserted just before
    the instruction; the sequencer executes them in order, so semantics are
    unchanged."""
    counter = 0
    for f in nc.m.functions:
        for bb in f.blocks:
            il = list(bb.instructions)
            out = []
            changed = False
            for inst in il:
                si = inst.sync_info
                if si is not None and si.on_wait and len(si.on_wait) > 1:
                    waits = list(si.on_wait)
                    extra, keep = waits[:-1], waits[-1:]
                    for w in extra:
                        nop = mybir.InstNoOp(
                            name=f"waitsplit-{counter}", engine=inst.engine,
                            ins=[], outs=[],
                            sync_info=mybir.SyncInfo(on_wait=[w], on_update=[]))
                        counter += 1
                        out.append(nop)
                    si.on_wait = keep
                    inst.sync_info = si
                    changed = True
                out.append(inst)
            if changed:
                bb.instructions = out
    return counter


def build(repeats: int = 1, split_waits: bool = True) -> bass.Bass:
    LABELS.clear()
    nc = bass.Bass()
    # const AP for the Act-relu bias (mirrors Bass.__init__'s registrations)
    eps_t = nc.alloc_sbuf_tensor("const-eps", [128, 1], f32)
    nc.gpsimd.memset(eps_t.ap(), KERNEL_EPS)
    nc.const_aps.aps[(f32, KERNEL_EPS)] = eps_t.ap()
    kw_d = nc.dram_tensor("kw", [128, 64 + 1024], bf16, kind="ExternalInput")
    qd_d = nc.dram_tensor("qd", [128, 1024], bf16, kind="ExternalInput")
    valt_d = nc.dram_tensor("valt", [2, 128, 8 * (V + 1)], bf16,
                            kind="ExternalInput")
    mask_d = nc.dram_tensor("mask4", [C, 4 * C], bf16, kind="ExternalInput")
    outt_d = nc.dram_tensor("outt", [2, 128, 2 * 4 * (V + 1)], bf16,
                            kind="ExternalOutput")

    mx = mybir.AluOpType.max
    ad = mybir.AluOpType.add
    ml = mybir.AluOpType.mult
    actRelu = mybir.ActivationFunctionType.Relu

    nc._tail_insts = []

    with _TileContextSplitDrain(nc) as tc:
        with (
            tc.tile_pool(name="io", bufs=1) as io,
            tc.tile_pool(name="big", bufs=4, space="PSUM") as big,
            tc.tile_pool(name="psKc", bufs=1, space="PSUM") as psKc,
            tc.tile_pool(name="psS", bufs=2, space="PSUM") as psS,
            tc.tile_pool(name="psO", bufs=1, space="PSUM") as psO,
            tc.tile_pool(name="sbA", bufs=2) as sbA,
            tc.tile_pool(name="sbK", bufs=2) as sbK,
            tc.tile_pool(name="sbS", bufs=2) as sbS,
        ):
            for _ in range(repeats):
                # ---- input tiles
                kw_s = io.tile([128, 64 + 1024], bf16, tag="kw")
                q_s = io.tile([128, 1024], bf16, tag="q")
                vt = {h: io.tile([128, 8 * (V + 1)], bf16, tag=f"v{h}", name=f"vt{h}")
                      for h in range(2)}
                mk_s = io.tile([C, 4 * C], bf16, tag="mk")
                # HWDGE queue: kw0, v0, q0, kw1, q1, v1
                _lab("dma_kw0", nc.sync.dma_start(kw_s[0:64, :], kw_d[0:64, :]))
                _lab("dma_v0", nc.sync.dma_start(vt[0][:], valt_d[0]))
                _lab("dma_q0", nc.sync.dma_start(q_s[0:64, :], qd_d[0:64, :]))
                _lab("dma_kw1", nc.sync.dma_start(kw_s[64:128, :], kw_d[64:128, :]))
                _lab("dma_q1", nc.sync.dma_start(q_s[64:128, :], qd_d[64:128, :]))
                _lab("dma_v1", nc.sync.dma_start(vt[1][:], valt_d[1]))
                # Pool SWDGE: mask
                _lab("dma_mk", nc.gpsimd.dma_start(mk_s[:], mask_d[:]))

                # ---- output staging (bf16, divided on host)
                o2 = {h: io.tile([128, 2 * 4 * (V + 1)], bf16, tag=f"o{h}", name=f"o2_{h}")
                      for h in range(2)}

                # phiqk[h]: cols [q 1024 | k 1024] bf16
                phiqk = {h: io.tile([M, 2048], bf16, tag=f"phiqk{h}", name=f"phiqk{h}")
                         for h in range(2)}

                def phi_piece(nm, h, g, u, eng):
                    """one (64,512) phi piece: g=0 -> q, g=1 -> k; u = piece.
                    eng in {'dve','pool','act'}; act uses relu(x+eps) approx."""
                    rows = slice(64 * h, 64 * h + 64)
                    wh = kw_s[rows, 0:64]
                    src = kw_s[rows, 64 + 512 * u:64 + 512 * u + 512] if g \
                        else q_s[rows, 512 * u:512 * u + 512]
                    pphi = big.tile([M, 512], f32, tag="big", name="pphi")
                    _lab(f"mm_phi_{nm}", nc.tensor.matmul(
                        pphi[:], lhsT=wh, rhs=src, start=True, stop=True))
                    dst = phiqk[h][:, 1024 * g + 512 * u:1024 * g + 512 * u + 512]
                    if eng == "act" and not ACT_RELU:
                        eng = "dve"
                    if eng == "act":
                        _lab(f"relu_{nm}", nc.scalar.activation(
                            dst, pphi[:], actRelu, bias=KERNEL_EPS))
                    else:
                        _lab(f"relu_{nm}", nc.vector.tensor_scalar(
                            dst, pphi[:], 0.0, KERNEL_EPS, op0=mx, op1=ad))

                S_cur = None

                def quad(t):
                    nonlocal S_cur
                    h, qd = t // 2, t % 2
                    rows = slice(64 * h, 64 * h + 64)
                    wh = kw_s[rows, 0:64]

                    # ---- phi_k in (C,M) layout: 4 chunk matmuls + one relu
                    pKc = psKc.tile([C, 4 * M], f32, tag="kc", name="pKc")
                    for j in range(4):
                        a = 4 * qd + j
                        _lab(f"mm_kc{t}_{j}", nc.tensor.matmul(
                            pKc[:, 64 * j:64 * j + 64],
                            lhsT=kw_s[rows, 64 + 128 * a:64 + 128 * a + 128],
                            rhs=wh, start=True, stop=True))
                    phiK = sbK.tile([C, 4 * M], bf16, tag="phiK", name="phiK")
                    if ACT_RELU:
                        _lab(f"relu_kc{t}", nc.scalar.activation(
                            phiK[:], pKc[:], actRelu, bias=KERNEL_EPS))
                    else:
                        _lab(f"relu_kc{t}", nc.vector.tensor_scalar(
                            phiK[:], pKc[:], 0.0, KERNEL_EPS, op0=mx, op1=ad))

                    # ---- A^T quad: 4 matmuls into one bank, one mask mult
                    pA = big.tile([C, 4 * C], f32, tag="big", name="pA")
                    for j in range(4):
                        a = 4 * qd + j
                        _lab(f"mm_A{t}_{j}", nc.tensor.matmul(
                            pA[:, 128 * j:128 * j + 128],
                            lhsT=phiqk[h][:, 1024 + 128 * a:1024 + 128 * a + 128],
                            rhs=phiqk[h][:, 128 * a:128 * a + 128],
                            start=True, stop=True))
                    At = sbA.tile([C, 4 * C], bf16, tag="At", name="At")
                    _lab(f"mask{t}", nc.vector.tensor_tensor(
                        At[:], pA[:], mk_s[:], op=ml))

                    # ---- dS quad: 4 matmuls into one psS bank.
                    # Pool cannot read PSUM on this HW, so the serial state
                    # prefix runs on Pool over an SBUF copy of the quad's dS
                    # (one DVE tensor_copy per quad); the last quad keeps
                    # per-chunk DVE adds to shorten the tail chain.
                    rows = slice(0, 64)
                    last = t == NQ - 1
                    dSq = psS.tile([128, 4 * (V + 1)], f32, tag="dS", name="dSq")
                    ndS = 3 if last else 4          # S_15 unused
                    for j in range(ndS):
                        i = 4 * t + j
                        a = 4 * qd + j
                        _lab(f"mm_dS{i}", nc.tensor.matmul(
                            dSq[rows, (V + 1) * j:(V + 1) * j + V + 1],
                            lhsT=phiK[:, 64 * j:64 * j + 64],
                            rhs=vt[h][:, (V + 1) * a:(V + 1) * a + V + 1],
                            start=True, stop=True))
                    dS_sb = None
                    if not last:
                        dS_sb = sbS.tile([128, 4 * (V + 1)], bf16, tag="dSsb",
                                         name="dS_sb")
                        _lab(f"dScp{t}", nc.vector.tensor_copy(
                            dS_sb[rows, :], dSq[rows, :]))

                    # state prefix + inter/intra per chunk
                    pO = psO.tile([C, 4 * (V + 1)], f32, tag="pO", name="pO")
                    S_list = []
                    for j in range(ndS):
                        i = 4 * t + j
                        prev = S_list[-1] if S_list else S_cur
                        if i == 0:
                            S_list.append(dS_sb)  # S_0 == dS_sb[:, 0:65]
                            continue
                        S_next = sbS.tile([128, V + 1], bf16, tag="S",
                                          name="S", bufs=3)
                        if last:
                            _lab(f"Sadd{i}", nc.vector.tensor_tensor(
                                S_next[rows, :], prev[rows, 0:V + 1],
                                dSq[rows, (V + 1) * j:(V + 1) * j + V + 1],
                                op=ad))
                        else:
                            _lab(f"Sadd{i}", nc.gpsimd.tensor_tensor(
                                S_next[rows, :], prev[rows, 0:V + 1],
                                dS_sb[rows, (V + 1) * j:(V + 1) * j + V + 1],
                                op=ad))
                        S_list.append(S_next)
                    for j in range(4):
                        i = 4 * t + j
                        a = 4 * qd + j
                        Vc = vt[h][:, (V + 1) * a:(V + 1) * a + V + 1]
                        oc = pO[:, (V + 1) * j:(V + 1) * j + V + 1]
                        if i > 0:
                            _lab(f"mm_inter{i}", nc.tensor.matmul(
                                oc, lhsT=phiqk[h][:, 128 * a:128 * a + 128],
                                rhs=S_cur[rows, 0:V + 1], start=True,
                                stop=False))
                        _lab(f"mm_intra{i}", nc.tensor.matmul(
                            oc, lhsT=At[:, 128 * j:128 * j + 128],
                            rhs=Vc, start=(i == 0), stop=True))
                        if j < ndS:
                            S_cur = S_list[j]

                    # ---- quad copy to staging + flush
                    W4 = 4 * (V + 1)
                    base = W4 * qd
                    if t < NQ - 1:
                        _lab(f"ocp{t}", nc.scalar.copy(
                            o2[h][:, base:base + W4], pO[:]))
                        di = _lab(f"dma_out{t}", nc.sync.dma_start(
                            outt_d[h, :, base:base + W4],
                            o2[h][:, base:base + W4]))
                        nc._tail_insts.append(di.ins)
                    else:
                        # split the last quad so only a (128,65) copy + DMA
                        # trail the final matmul
                        W3 = 3 * (V + 1)
                        _lab(f"ocp{t}a", nc.scalar.copy(
                            o2[h][:, base:base + W3], pO[:, 0:W3]))
                        di = _lab(f"dma_out{t}a", nc.sync.dma_start(
                            outt_d[h, :, base:base + W3],
                            o2[h][:, base:base + W3]))
                        nc._tail_insts.append(di.ins)
                        _lab(f"ocp{t}b", nc.scalar.copy(
                            o2[h][:, base + W3:base + W4], pO[:, W3:W4]))
                        di = _lab(f"dma_out{t}b", nc.sync.dma_start(
                            outt_d[h, :, base + W3:base + W4],
                            o2[h][:, base + W3:base + W4]))
                        nc._tail_insts.append(di.ins)

                # ---- emission order (drives PSUM big-pool rotation, bufs=4)
                phi_piece("k0a", 0, 1, 0, "dve")
                phi_piece("q0a", 0, 0, 0, "dve")
                phi_piece("q0b", 0, 0, 1, "act")
                quad(0)
                phi_piece("k0b", 0, 1, 1, "dve")
                phi_piece("k1a", 1, 1, 0, "act")
                quad(1)
                phi_piece("k1b", 1, 1, 1, "act")
                phi_piece("q1a", 1, 0, 0, "act")
                quad(2)
                phi_piece("q1b", 1, 0, 1, "act")
                quad(3)

    if split_waits:
        _split_instruction_waits(nc)
    return nc


_CONSTS = None


def _consts():
    global _CONSTS
    if _CONSTS is None:
        masku = np.triu(np.ones((C, C), dtype=np.float32))
        mask4 = np.tile(masku, (1, 4))
        _CONSTS = mask4
    return _CONSTS


def kernel(keys, values, queries, proj_matrix):
    import ml_dtypes
    bf = ml_dtypes.bfloat16
    keys = np.asarray(keys, dtype=np.float32)
    queries = np.asarray(queries, dtype=np.float32)
    proj_matrix = np.asarray(proj_matrix, dtype=np.float32)

    # kw: rows 0:64 = [W | k cols 0:1024]; rows 64:128 = [W | k cols 1024:]
    kw = np.empty((B, 128, 64 + 1024), dtype=np.float32)
    kw[:, 0:64, 0:64] = proj_matrix[None]
    kw[:, 64:128, 0:64] = proj_matrix[None]
    kw[:, 0:64, 64:] = keys[:, :, 0:1024]
    kw[:, 64:128, 64:] = keys[:, :, 1024:2048]
    qd = np.empty((B, 128, 1024), dtype=np.float32)
    qd[:, 0:64, :] = queries[:, :, 0:1024]
    qd[:, 64:128, :] = queries[:, :, 1024:2048]

    vT = np.asarray(values, dtype=np.float32).transpose(0, 2, 1)  # (B, L, V)
    vT = vT.reshape(B, 2, 8, 128, V).transpose(0, 1, 3, 2, 4)  # (B,2,128,8,V)
    valt = np.ones((B, 2, 128, 8, V + 1), dtype=np.float32)
    valt[..., 0:V] = vT
    valt = valt.reshape(B, 2, 128, 8 * (V + 1))
    mask4 = _consts()

    kw = np.ascontiguousarray(kw.astype(bf))
    qd = np.ascontiguousarray(qd.astype(bf))
    valt = np.ascontiguousarray(valt.astype(bf))
    mask4 = np.ascontiguousarray(mask4.astype(bf))

    nc = build()
    in_maps = [
        {"kw": kw[b], "qd": qd[b], "valt": valt[b], "mask4": mask4}
        for b in range(B)
    ]
    res = run_bass_kernel_spmd(nc, in_maps, list(range(NCORES)))
    # outt (2, 128, 520) bf16: [h, p, 260*qd + 65*j + c]; c=64 is the norm.
    # position l = 1024h + 512qd + 128j + p
    outs = []
    for b in range(B):
        ot = np.asarray(res.results[b]["outt"], dtype=np.float32)
        ot = ot.reshape(2, 128, 2, 4, V + 1)        # (h, p, qd, j, c)
        num, den = ot[..., 0:V], ot[..., V:V + 1]
        o = num / den                               # (h, p, qd, j, v)
        o = o.transpose(4, 0, 2, 3, 1).reshape(V, L)
        outs.append(o)
    return np.ascontiguousarray(np.stack(outs, axis=0), dtype=np.float32)


if __name__ == "__main__":
    rng = np.random.default_rng(0)
    ks = rng.standard_normal((B, D, L), dtype=np.float32)
    vs = rng.standard_normal((B, V, L), dtype=np.float32)
    qs = rng.standard_normal((B, D, L), dtype=np.float32)
    pm = np.linalg.qr(rng.standard_normal((D, M)))[0].astype(np.float32)
    o = kernel(ks, vs, qs, pm)
    print("kernel output", o.shape, o.dtype)
